# revision 1
# baseline (speedup 1.0000x reference)
"""HSTU-style attention block (RoPE + multi-scale temporal agg + SDPA + LN + out-proj)
for Trainium2, data-parallel over batch across 8 NeuronCores.

Per-core layout strategy (batch element per core):
  - host pre-transposes X so projections run with activations as lhsT
  - Q/K/V projected into natural [s, h'] layout; RoPE applied with strided DVE ops
  - temporal aggregation applied as a matmul against a host-built [S, S] matrix T
    (derived from softmax(temporal_weights)); Q/K produced transposed, V natural
    with an extra ones column so softmax denominators ride the PV matmul
  - attention computes scores^T per head (contraction zero-padded to K=128 to
    keep the PE clock warm), Exp on the scalar engine, PV accumulation over key
    chunks; softmax normalization + LayerNorm statistics fold into the head loop
  - LayerNorm runs across partitions (PE ones-matmul sums), out-projection natural
All matmuls run in float32r (TF32-like precision, ~2x bf16 cycle cost).
"""

import numpy as np
import concourse.mybir as mybir
import concourse.tile as tile
from concourse import bacc
from concourse.bass_utils import run_bass_kernel_spmd

B, S, H, NH = 8, 1024, 1024, 16
HD = H // NH  # 64
P = 128
SO = S // P  # 8
HO = H // P  # 8
N_SCALES = 4
LN_EPS = 1e-5
F32 = mybir.dt.float32
F32R = mybir.dt.float32r

N_CORES = 8


# ---------------------------------------------------------------- host helpers
def _softmax_np(x):
    x = np.asarray(x, np.float64)
    e = np.exp(x - x.max())
    return e / e.sum()


def _temporal_matrix(temporal_weights):
    """[S, S] matrix T with (T @ x) == temporal_agg(x) along the sequence axis."""
    w = _softmax_np(temporal_weights)
    T = np.eye(S, dtype=np.float64) * w[0]
    for scale in range(1, N_SCALES):
        p = max(1, S // (2 ** scale))
        k = S // p
        pool = np.zeros((p, S), dtype=np.float64)
        for j in range(p):
            pool[j, j * k:(j + 1) * k] = 1.0 / k
        coord = (np.arange(S, dtype=np.float64) + 0.5) * (p / S) - 0.5
        coord = np.clip(coord, 0.0, None)
        i0 = np.minimum(np.floor(coord).astype(np.int64), p - 1)
        i1 = np.minimum(i0 + 1, p - 1)
        lam = (coord - i0).astype(np.float32).astype(np.float64)
        interp = np.zeros((S, p), dtype=np.float64)
        interp[np.arange(S), i0] += 1.0 - lam
        interp[np.arange(S), i1] += lam
        T += w[scale] * (interp @ pool)
    return T.astype(np.float32)


def _rope_tables():
    inv_freq = 1.0 / (10000.0 ** (np.arange(0, HD, 2, dtype=np.float64) / HD))
    freqs = np.arange(S, dtype=np.float64)[:, None] * inv_freq[None, :]
    cos = np.repeat(np.cos(freqs), 2, axis=-1).astype(np.float32)  # [S, HD]
    sin = np.repeat(np.sin(freqs), 2, axis=-1).astype(np.float32)
    return cos, sin


def _nat(x):
    """[S, D] -> [P, S//P, D] with x[so*P+p, d] = out[p, so, d]."""
    return np.ascontiguousarray(x.reshape(SO, P, x.shape[-1]).transpose(1, 0, 2))


def _xt_chunks(x):
    """[S, H] -> [P, SO, HO*P] with out[p, so, ho*P + i] = x[so*P + i, ho*P + p]."""
    return np.ascontiguousarray(
        x.reshape(SO, P, HO, P).transpose(3, 0, 2, 1).reshape(P, SO, H))


# ---------------------------------------------------------------- bass program
def _build_program():
    nc = bacc.Bacc("TRN2", target_bir_lowering=False, debug=False)

    d_xt = {a: nc.dram_tensor(f"xt_{a}", [P, SO, H], F32R, kind="ExternalInput")
            for a in ("v", "q", "k")}
    d_w = {a: nc.dram_tensor(f"w_{a}", [P, HO, H], F32R, kind="ExternalInput")
           for a in ("v", "q", "k", "o")}
    d_b = {a: nc.dram_tensor(f"b_{a}", [1, H], F32, kind="ExternalInput")
           for a in ("v", "q", "k", "o")}
    d_tt = nc.dram_tensor("tt", [P, SO, S], F32R, kind="ExternalInput")
    d_cos = nc.dram_tensor("cos_t", [P, SO, HD], F32, kind="ExternalInput")
    d_sin = nc.dram_tensor("sin_t", [P, SO, HD], F32, kind="ExternalInput")
    d_gam = nc.dram_tensor("ln_g", [P, HO], F32, kind="ExternalInput")
    d_bet = nc.dram_tensor("ln_b", [P, HO], F32, kind="ExternalInput")
    d_y = nc.dram_tensor("y", [P, SO, H], F32, kind="ExternalOutput")
    d_zp = nc.dram_tensor("zpad", [HD, S], F32R, kind="ExternalInput")
    # per-chunk scratch so a head's reload only waits on its own spill DMA
    d_qs = [nc.dram_tensor(f"q_scr{hc}", [P, S], F32R) for hc in range(HO)]
    d_ks = [nc.dram_tensor(f"k_scr{hc}", [P, S], F32R) for hc in range(HO)]

    with tile.TileContext(nc) as tc:
        with (
            tc.tile_pool(name="const", bufs=1) as cpool,
            tc.tile_pool(name="big", bufs=4) as big,
            tc.tile_pool(name="s4", bufs=10) as s4,
            tc.tile_pool(name="s2", bufs=6) as s2,
            tc.tile_pool(name="mm_ps", bufs=4, space="PSUM") as mmps,
            tc.tile_pool(name="sc_ps", bufs=2, space="PSUM") as scps,
            tc.tile_pool(name="pv_ps", bufs=2, space="PSUM") as pvps,
        ):
            cos_t = cpool.tile([P, SO, HD], F32, name="cos_t")
            sin_t = cpool.tile([P, SO, HD], F32, name="sin_t")
            nc.sync.dma_start(cos_t[:], d_cos.ap())
            nc.sync.dma_start(sin_t[:], d_sin.ap())
            gam_t = cpool.tile([P, HO], F32, name="gam_t")
            bet_t = cpool.tile([P, HO], F32, name="bet_t")
            nc.sync.dma_start(gam_t[:], d_gam.ap())
            nc.sync.dma_start(bet_t[:], d_bet.ap())
            ones = cpool.tile([P, 1], F32, name="ones")
            nc.vector.memset(ones[:], 1.0)
            eps_t = cpool.tile([P, 1], F32, name="eps_t")
            nc.vector.memset(eps_t[:], LN_EPS)

            def _rope_chunk(a_nat, so):
                ch = a_nat[:, so, :]
                ch3 = ch.rearrange("p (nh d) -> p nh d", d=HD)
                ch4 = ch.rearrange("p (nh hf dd) -> p nh hf dd", hf=2, dd=HD // 2)
                rot = s4.tile([P, H], F32, tag="s4")
                rot4 = rot[:].rearrange("p (nh hf dd) -> p nh hf dd",
                                        hf=2, dd=HD // 2)
                rot3 = rot[:].rearrange("p (nh d) -> p nh d", d=HD)
                nc.vector.tensor_scalar_mul(rot4[:, :, 0, :], ch4[:, :, 1, :], -1.0)
                nc.vector.tensor_copy(rot4[:, :, 1, :], ch4[:, :, 0, :])
                cb = cos_t[:, so, :][:, None, :].to_broadcast((P, NH, HD))
                sb = sin_t[:, so, :][:, None, :].to_broadcast((P, NH, HD))
                nc.vector.tensor_tensor(ch3[:], ch3[:], cb, mybir.AluOpType.mult)
                nc.vector.tensor_tensor(rot3[:], rot3[:], sb, mybir.AluOpType.mult)
                nc.vector.tensor_tensor(ch[:], ch[:], rot[:], mybir.AluOpType.add)

            def project(a, do_rope=False):
                """A_nat [P, SO, H] (f32r) = X @ W_a + b_a, optional fused RoPE.

                RoPE is interleaved per s-chunk so DVE work tracks PE work and
                the tensor engine never idles long enough to drop its clock."""
                w_t = big.tile([P, HO, H], F32R, tag="big")
                nc.sync.dma_start(w_t[:], d_w[a].ap())
                brow = s4.tile([1, H], F32, tag="s4")
                nc.sync.dma_start(brow[:], d_b[a].ap())
                bb = s4.tile([P, H], F32, tag="s4")
                nc.gpsimd.partition_broadcast(bb[:], brow[:])
                a_nat = big.tile([P, SO, H], F32R, tag="big")
                for so in range(SO):
                    xt_c = s4.tile([P, HO, P], F32R, tag="s4")
                    nc.sync.dma_start(xt_c[:], d_xt[a].ap()[:, so, :])
                    for nh in range(2):
                        ps = mmps.tile([P, 512], F32, tag="mm")
                        for ko in range(HO):
                            nc.tensor.matmul(
                                ps[:], xt_c[:, ko, :],
                                w_t[:, ko, nh * 512:(nh + 1) * 512],
                                start=(ko == 0), stop=(ko == HO - 1))
                        nc.vector.tensor_tensor(
                            a_nat[:, so, nh * 512:(nh + 1) * 512], ps[:],
                            bb[:, nh * 512:(nh + 1) * 512], mybir.AluOpType.add)
                    if do_rope:
                        _rope_chunk(a_nat, so)
                return a_nat

            BAND = 12  # T[s', s] == 0 for |s' - s| > 11 (structural)

            def _band_sos(o0, o1):
                """so chunks whose s-range intersects [o0-BAND, o1+BAND)."""
                return [so for so in range(SO)
                        if so * P + P > o0 - BAND and so * P < o1 + BAND]

            def t_agg_spill(a_nat, tt, d_scr):
                """(T @ A).T evicted through SBUF chunks into DRAM scratch.
                Evictions ride the scalar engine -- idle during phase 1."""
                for hc in range(HO):
                    for sh in range(2):
                        sos = _band_sos(sh * 512, (sh + 1) * 512)
                        ps = mmps.tile([P, 512], F32, tag="mm")
                        for so in sos:
                            nc.tensor.matmul(
                                ps[:], a_nat[:, so, hc * P:(hc + 1) * P],
                                tt[:, so, sh * 512:(sh + 1) * 512],
                                start=(so == sos[0]), stop=(so == sos[-1]))
                        ev = s2.tile([P, 512], F32R, tag="s2")
                        nc.scalar.copy(ev[:], ps[:])
                        nc.sync.dma_start(
                            d_scr[hc].ap()[:, sh * 512:(sh + 1) * 512], ev[:])

            def t_agg_v(v_nat, tt):
                """V_ext [P, SO, NH, HD+1] (f32r) = T @ V with ones column."""
                v_ext = big.tile([P, SO, NH, HD + 1], F32R, tag="big")
                nc.vector.tensor_copy(
                    v_ext[:, :, :, HD:HD + 1],
                    ones[:, None, None, :].to_broadcast((P, SO, NH, 1)))
                for sc in range(SO):
                    sos = _band_sos(sc * P, (sc + 1) * P)
                    for dh in range(2):
                        ps = mmps.tile([P, 512], F32, tag="mm")
                        for so in sos:
                            nc.tensor.matmul(
                                ps[:], tt[:, so, sc * P:(sc + 1) * P],
                                v_nat[:, so, dh * 512:(dh + 1) * 512],
                                start=(so == sos[0]), stop=(so == sos[-1]))
                        pvw = ps[:].rearrange("p (nh d) -> p nh d", d=HD)
                        nc.scalar.copy(
                            v_ext[:, sc, dh * 8:(dh + 1) * 8, 0:HD], pvw)
                return v_ext

            # ---- phase 1: V, Q, K  (projection + RoPE + temporal aggregation)
            v_nat = project("v")
            tt = big.tile([P, SO, S], F32R, tag="big")
            nc.sync.dma_start(tt[:], d_tt.ap())
            v_ext = t_agg_v(v_nat, tt)

            q_nat = project("q", do_rope=True)
            t_agg_spill(q_nat, tt, d_qs)

            k_nat = project("k", do_rope=True)
            t_agg_spill(k_nat, tt, d_ks)

            # ---- phase 2: attention (normalization + LN stats fold into the
            # head loop so the tail barrier shrinks to the final LN apply)
            attn_T = big.tile([P, HO, S], F32, tag="big")
            acc = s4.tile([P, S], F32R, tag="s4")
            acc2 = s4.tile([P, S], F32R, tag="s4")
            rb_c = None
            for h in range(NH):
                hc, off = h // 2, (h % 2) * HD
                # zero-pad the contraction dim to K=128: half-array (K=64)
                # matmuls never trip the PE activity monitor, pinning the
                # clock at 1.2 GHz. Rows 64:128 come from a DRAM zeros pad.
                kh = s4.tile([P, S], F32R, tag="s4")
                nc.sync.dma_start(kh[0:HD, :], d_ks[hc].ap()[off:off + HD, :])
                nc.sync.dma_start(kh[HD:P, :], d_zp.ap())
                qh = s4.tile([P, S], F32R, tag="s4")
                nc.sync.dma_start(qh[0:HD, :], d_qs[hc].ap()[off:off + HD, :])
                nc.sync.dma_start(qh[HD:P, :], d_zp.ap())
                if off == 0:
                    rb_c = s4.tile([P, S], F32, tag="s4")
                for q2 in range(2):
                    pv = pvps.tile([P, 512], F32, tag="pv")
                    # software-pipelined: scores run one kc ahead of PV
                    ets = []
                    for kc in range(SO + 1):
                        if kc < SO:
                            sp = scps.tile([P, 512], F32, tag="sc")
                            nc.tensor.matmul(
                                sp[:], kh[0:P, kc * P:(kc + 1) * P],
                                qh[0:P, q2 * 512:(q2 + 1) * 512],
                                start=True, stop=True, skip_group_check=True)
                            e_t = s2.tile([P, 512], F32R, tag="s2")
                            nc.scalar.activation(
                                e_t[:], sp[:],
                                mybir.ActivationFunctionType.Exp, scale=0.125)
                            ets.append(e_t)
                        if kc > 0:
                            j = kc - 1
                            nc.tensor.matmul(
                                pv[0:HD + 1, :], v_ext[:, j, h, :], ets[j][:],
                                start=(j == 0), stop=(j == SO - 1),
                                skip_group_check=True)
                    # evict raw out + sums; broadcast sums (no PE dependency).
                    # partition_broadcast only writes reliably at partition 0,
                    # so odd heads bounce through a temp + DVE copy.
                    qs = slice(q2 * 512, (q2 + 1) * 512)
                    nc.vector.tensor_copy(attn_T[off:off + HD, hc, qs], pv[0:HD, :])
                    srow = s2.tile([1, 512], F32, tag="s2")
                    nc.vector.tensor_copy(srow[:], pv[HD:HD + 1, :])
                    if off == 0:
                        nc.gpsimd.partition_broadcast(rb_c[0:HD, qs], srow[:])
                    else:
                        tmp = s2.tile([HD, 512], F32, tag="s2")
                        nc.gpsimd.partition_broadcast(tmp[:], srow[:])
                        nc.vector.tensor_copy(rb_c[off:off + HD, qs], tmp[:])
                if off == HD:
                    # chunk hc complete: normalize + accumulate LN stats
                    rcp_c = s4.tile([P, S], F32, tag="s4")
                    nc.vector.reciprocal_approx_fast(rcp_c[:], rb_c[:])
                    nc.vector.tensor_tensor(attn_T[:, hc, :], attn_T[:, hc, :],
                                            rcp_c[:], mybir.AluOpType.mult)
                    if hc == 0:
                        nc.vector.tensor_copy(acc[:], attn_T[:, 0, :])
                        nc.vector.tensor_tensor(acc2[:], attn_T[:, 0, :],
                                                attn_T[:, 0, :],
                                                mybir.AluOpType.mult)
                    else:
                        nc.vector.tensor_tensor(acc[:], acc[:], attn_T[:, hc, :],
                                                mybir.AluOpType.add)
                        sqc = s4.tile([P, S], F32, tag="s4")
                        nc.vector.tensor_tensor(sqc[:], attn_T[:, hc, :],
                                                attn_T[:, hc, :],
                                                mybir.AluOpType.mult)
                        nc.vector.tensor_tensor(acc2[:], acc2[:], sqc[:],
                                                mybir.AluOpType.add)

            # prefetch out-projection weights so the DMA overlaps LayerNorm
            wo_t = big.tile([P, HO, H], F32R, tag="big")
            nc.sync.dma_start(wo_t[:], d_w["o"].ap())
            brow_o = s4.tile([1, H], F32, tag="s4")
            nc.sync.dma_start(brow_o[:], d_b["o"].ap())
            bo_b = s4.tile([P, H], F32, tag="s4")
            nc.gpsimd.partition_broadcast(bo_b[:], brow_o[:])

            # ---- phase 3: LayerNorm over h (partition axis across HO chunks)
            # partition sums via a PE ones-matmul (gpsimd allreduce is slow)
            ones_r = cpool.tile([P, 1], F32R, name="ones_r")
            nc.vector.tensor_copy(ones_r[:], ones[:])
            mu_b = s4.tile([P, S], F32, tag="s4")
            ms_b = s4.tile([P, S], F32, tag="s4")
            for src, dst in ((acc, mu_b), (acc2, ms_b)):
                for half in range(2):
                    pss = mmps.tile([P, 512], F32, tag="mm")
                    nc.tensor.matmul(pss[0:1, :], ones_r[:],
                                     src[:, half * 512:(half + 1) * 512],
                                     start=True, stop=True,
                                     skip_group_check=True)
                    srw = s2.tile([1, 512], F32, tag="s2")
                    nc.vector.tensor_copy(srw[:], pss[0:1, :])
                    nc.gpsimd.partition_broadcast(
                        dst[:, half * 512:(half + 1) * 512], srw[:])
            nc.vector.tensor_scalar_mul(mu_b[:], mu_b[:], 1.0 / H)
            nc.vector.tensor_scalar_mul(ms_b[:], ms_b[:], 1.0 / H)
            m2 = s4.tile([P, S], F32, tag="s4")
            nc.scalar.square(m2[:], mu_b[:])
            nc.vector.tensor_tensor(ms_b[:], ms_b[:], m2[:], mybir.AluOpType.subtract)
            nc.scalar.activation(ms_b[:], ms_b[:], mybir.ActivationFunctionType.Sqrt,
                                 bias=eps_t[:])
            rstd = s4.tile([P, S], F32, tag="s4")
            nc.vector.reciprocal_approx_fast(rstd[:], ms_b[:])

            ln_out = big.tile([P, HO, S], F32R, tag="big")
            for hc in range(HO):
                t1 = s4.tile([P, S], F32, tag="s4")
                nc.vector.tensor_tensor(t1[:], attn_T[:, hc, :], mu_b[:],
                                        mybir.AluOpType.subtract)
                nc.vector.tensor_tensor(t1[:], t1[:], rstd[:],
                                        mybir.AluOpType.mult)
                nc.vector.tensor_scalar(ln_out[:, hc, :], t1[:],
                                        gam_t[:, hc:hc + 1], bet_t[:, hc:hc + 1],
                                        mybir.AluOpType.mult, mybir.AluOpType.add)

            # ---- phase 4: output projection
            for so in range(SO):
                for nh in range(2):
                    ps = mmps.tile([P, 512], F32, tag="mm")
                    for hc in range(HO):
                        nc.tensor.matmul(
                            ps[:], ln_out[:, hc, so * P:(so + 1) * P],
                            wo_t[:, hc, nh * 512:(nh + 1) * 512],
                            start=(hc == 0), stop=(hc == HO - 1))
                    ych = s2.tile([P, 512], F32, tag="s2")
                    nc.vector.tensor_tensor(ych[:], ps[:],
                                            bo_b[:, nh * 512:(nh + 1) * 512],
                                            mybir.AluOpType.add)
                    nc.sync.dma_start(
                        d_y.ap()[:, so, nh * 512:(nh + 1) * 512], ych[:])

    nc.compile()
    return nc


_NC = None


def _get_nc():
    global _NC
    if _NC is None:
        _NC = _build_program()
    return _NC


def _host_inputs(query, key, value, Wq, bq, Wk, bk, Wv, bv, Wo, bo,
                 temporal_weights, ln_gamma, ln_beta):
    T = _temporal_matrix(temporal_weights)
    tt_host = np.ascontiguousarray(  # TT[p, so, s'] = T[s', so*P+p]
        T.T.reshape(SO, P, S).transpose(1, 0, 2))
    cos, sin = _rope_tables()
    common = {
        "w_v": _nat(np.asarray(Wv, np.float32)),
        "w_q": _nat(np.asarray(Wq, np.float32)),
        "w_k": _nat(np.asarray(Wk, np.float32)),
        "w_o": _nat(np.asarray(Wo, np.float32)),
        "b_v": np.asarray(bv, np.float32).reshape(1, H),
        "b_q": np.asarray(bq, np.float32).reshape(1, H),
        "b_k": np.asarray(bk, np.float32).reshape(1, H),
        "b_o": np.asarray(bo, np.float32).reshape(1, H),
        "tt": tt_host,
        "zpad": np.zeros((HD, S), np.float32),
        "cos_t": _nat(cos),
        "sin_t": _nat(sin),
        "ln_g": np.ascontiguousarray(
            np.asarray(ln_gamma, np.float32).reshape(HO, P).T),
        "ln_b": np.ascontiguousarray(
            np.asarray(ln_beta, np.float32).reshape(HO, P).T),
    }
    in_maps = []
    for c in range(N_CORES):
        m = dict(common)
        m["xt_q"] = _xt_chunks(np.asarray(query[c], np.float32))
        m["xt_k"] = _xt_chunks(np.asarray(key[c], np.float32))
        m["xt_v"] = _xt_chunks(np.asarray(value[c], np.float32))
        in_maps.append(m)
    return in_maps


def kernel(query, key, value, Wq, bq, Wk, bk, Wv, bv, Wo, bo,
           temporal_weights, ln_gamma, ln_beta):
    in_maps = _host_inputs(query, key, value, Wq, bq, Wk, bk, Wv, bv, Wo, bo,
                           temporal_weights, ln_gamma, ln_beta)
    nc = _get_nc()
    res = run_bass_kernel_spmd(nc, in_maps, list(range(N_CORES)))
    out = np.empty((B, S, H), np.float32)
    for c in range(N_CORES):
        y = res.results[c]["y"]  # [P, SO, H]
        out[c] = y.transpose(1, 0, 2).reshape(S, H)
    return out



# revision 3
# speedup vs baseline: 1.0817x; 1.0817x over previous
"""HSTU-style attention block (RoPE + multi-scale temporal agg + SDPA + LN + out-proj)
for Trainium2, data-parallel over batch across 8 NeuronCores.

Per-core layout strategy (batch element per core):
  - host pre-transposes X so projections run with activations as lhsT
  - Q/K/V projected into natural [s, h'] layout; RoPE applied with strided DVE ops
  - temporal aggregation applied as a matmul against a host-built [S, S] matrix T
    (derived from softmax(temporal_weights)); Q/K produced transposed, V natural
    with an extra ones column so softmax denominators ride the PV matmul
  - attention computes scores^T per head (contraction zero-padded to K=128 to
    keep the PE clock warm), Exp on the scalar engine, PV accumulation over key
    chunks; softmax normalization + LayerNorm statistics fold into the head loop
  - LayerNorm runs across partitions (PE ones-matmul sums), out-projection natural
All matmuls run in bfloat16 (fp32 PSUM accumulation); rel-err budget is 2e-2 and
bf16 lands ~7e-3, while halving PE cycle cost vs float32r.
"""

import numpy as np
import ml_dtypes
import concourse.mybir as mybir
import concourse.tile as tile
from concourse import bacc
from concourse.bass_utils import run_bass_kernel_spmd

B, S, H, NH = 8, 1024, 1024, 16
HD = H // NH  # 64
P = 128
SO = S // P  # 8
HO = H // P  # 8
N_SCALES = 4
LN_EPS = 1e-5
F32 = mybir.dt.float32
F32R = mybir.dt.float32r
BF16 = mybir.dt.bfloat16
NPBF16 = ml_dtypes.bfloat16

N_CORES = 8


# ---------------------------------------------------------------- host helpers
def _softmax_np(x):
    x = np.asarray(x, np.float64)
    e = np.exp(x - x.max())
    return e / e.sum()


def _temporal_matrix(temporal_weights):
    """[S, S] matrix T with (T @ x) == temporal_agg(x) along the sequence axis."""
    w = _softmax_np(temporal_weights)
    T = np.eye(S, dtype=np.float64) * w[0]
    for scale in range(1, N_SCALES):
        p = max(1, S // (2 ** scale))
        k = S // p
        pool = np.zeros((p, S), dtype=np.float64)
        for j in range(p):
            pool[j, j * k:(j + 1) * k] = 1.0 / k
        coord = (np.arange(S, dtype=np.float64) + 0.5) * (p / S) - 0.5
        coord = np.clip(coord, 0.0, None)
        i0 = np.minimum(np.floor(coord).astype(np.int64), p - 1)
        i1 = np.minimum(i0 + 1, p - 1)
        lam = (coord - i0).astype(np.float32).astype(np.float64)
        interp = np.zeros((S, p), dtype=np.float64)
        interp[np.arange(S), i0] += 1.0 - lam
        interp[np.arange(S), i1] += lam
        T += w[scale] * (interp @ pool)
    return T.astype(np.float32)


def _rope_tables():
    inv_freq = 1.0 / (10000.0 ** (np.arange(0, HD, 2, dtype=np.float64) / HD))
    freqs = np.arange(S, dtype=np.float64)[:, None] * inv_freq[None, :]
    cos = np.repeat(np.cos(freqs), 2, axis=-1).astype(np.float32)  # [S, HD]
    sin = np.repeat(np.sin(freqs), 2, axis=-1).astype(np.float32)
    return cos, sin


def _nat(x):
    """[S, D] -> [P, S//P, D] with x[so*P+p, d] = out[p, so, d]."""
    return np.ascontiguousarray(x.reshape(SO, P, x.shape[-1]).transpose(1, 0, 2))


def _xt_chunks(x):
    """[S, H] -> [P, SO, HO*P] with out[p, so, ho*P + i] = x[so*P + i, ho*P + p]."""
    return np.ascontiguousarray(
        x.reshape(SO, P, HO, P).transpose(3, 0, 2, 1).reshape(P, SO, H))


# ---------------------------------------------------------------- bass program
def _build_program():
    nc = bacc.Bacc("TRN2", target_bir_lowering=False, debug=False)

    d_xt = {a: nc.dram_tensor(f"xt_{a}", [P, SO, H], BF16, kind="ExternalInput")
            for a in ("v", "q", "k")}
    d_w = {a: nc.dram_tensor(f"w_{a}", [P, HO, H], BF16, kind="ExternalInput")
           for a in ("v", "q", "k", "o")}
    d_b = {a: nc.dram_tensor(f"b_{a}", [1, H], F32, kind="ExternalInput")
           for a in ("v", "q", "k", "o")}
    d_tt = nc.dram_tensor("tt", [P, SO, S], BF16, kind="ExternalInput")
    d_cos = nc.dram_tensor("cos_t", [P, SO, HD], F32, kind="ExternalInput")
    d_sin = nc.dram_tensor("sin_t", [P, SO, HD], F32, kind="ExternalInput")
    d_gam = nc.dram_tensor("ln_g", [P, HO], F32, kind="ExternalInput")
    d_bet = nc.dram_tensor("ln_b", [P, HO], F32, kind="ExternalInput")
    d_y = nc.dram_tensor("y", [P, SO, H], F32, kind="ExternalOutput")
    d_zp = nc.dram_tensor("zpad", [HD, S], BF16, kind="ExternalInput")
    # per-chunk scratch so a head's reload only waits on its own spill DMA
    d_qs = [nc.dram_tensor(f"q_scr{hc}", [P, S], BF16) for hc in range(HO)]
    d_ks = [nc.dram_tensor(f"k_scr{hc}", [P, S], BF16) for hc in range(HO)]

    with tile.TileContext(nc) as tc:
        with (
            tc.tile_pool(name="const", bufs=1) as cpool,
            tc.tile_pool(name="big", bufs=4) as big,
            tc.tile_pool(name="s4", bufs=10) as s4,
            tc.tile_pool(name="s2", bufs=6) as s2,
            tc.tile_pool(name="mm_ps", bufs=4, space="PSUM") as mmps,
            tc.tile_pool(name="sc_ps", bufs=2, space="PSUM") as scps,
            tc.tile_pool(name="pv_ps", bufs=2, space="PSUM") as pvps,
        ):
            cos_t = cpool.tile([P, SO, HD], F32, name="cos_t")
            sin_t = cpool.tile([P, SO, HD], F32, name="sin_t")
            nc.sync.dma_start(cos_t[:], d_cos.ap())
            nc.sync.dma_start(sin_t[:], d_sin.ap())
            gam_t = cpool.tile([P, HO], F32, name="gam_t")
            bet_t = cpool.tile([P, HO], F32, name="bet_t")
            nc.sync.dma_start(gam_t[:], d_gam.ap())
            nc.sync.dma_start(bet_t[:], d_bet.ap())
            ones = cpool.tile([P, 1], F32, name="ones")
            nc.vector.memset(ones[:], 1.0)
            eps_t = cpool.tile([P, 1], F32, name="eps_t")
            nc.vector.memset(eps_t[:], LN_EPS)

            def _rope_chunk(st, so):
                """In-place RoPE on an f32 staging tile st [P, H]."""
                ch = st[:]
                ch3 = ch.rearrange("p (nh d) -> p nh d", d=HD)
                ch4 = ch.rearrange("p (nh hf dd) -> p nh hf dd", hf=2, dd=HD // 2)
                rot = s4.tile([P, H], F32, tag="s4")
                rot4 = rot[:].rearrange("p (nh hf dd) -> p nh hf dd",
                                        hf=2, dd=HD // 2)
                rot3 = rot[:].rearrange("p (nh d) -> p nh d", d=HD)
                nc.vector.tensor_scalar_mul(rot4[:, :, 0, :], ch4[:, :, 1, :], -1.0)
                nc.vector.tensor_copy(rot4[:, :, 1, :], ch4[:, :, 0, :])
                cb = cos_t[:, so, :][:, None, :].to_broadcast((P, NH, HD))
                sb = sin_t[:, so, :][:, None, :].to_broadcast((P, NH, HD))
                nc.vector.tensor_tensor(ch3[:], ch3[:], cb, mybir.AluOpType.mult)
                nc.vector.tensor_tensor(rot3[:], rot3[:], sb, mybir.AluOpType.mult)
                nc.vector.tensor_tensor(ch[:], ch[:], rot[:], mybir.AluOpType.add)

            def project(a, do_rope=False):
                """A_nat [P, SO, H] (bf16) = X @ W_a + b_a, optional fused RoPE.

                RoPE runs on an f32 staging chunk (single bf16 rounding at the
                end), interleaved per s-chunk so DVE work tracks PE work."""
                w_t = big.tile([P, HO, H], BF16, tag="big")
                nc.sync.dma_start(w_t[:], d_w[a].ap())
                brow = s4.tile([1, H], F32, tag="s4")
                nc.sync.dma_start(brow[:], d_b[a].ap())
                bb = s4.tile([P, H], F32, tag="s4")
                nc.gpsimd.partition_broadcast(bb[:], brow[:])
                a_nat = big.tile([P, SO, H], BF16, tag="big")
                for so in range(SO):
                    xt_c = s4.tile([P, HO, P], BF16, tag="s4")
                    nc.sync.dma_start(xt_c[:], d_xt[a].ap()[:, so, :])
                    st = (s4.tile([P, H], F32, tag="s4", name="st")
                          if do_rope else None)
                    for nh in range(2):
                        ps = mmps.tile([P, 512], F32, tag="mm")
                        for ko in range(HO):
                            nc.tensor.matmul(
                                ps[:], xt_c[:, ko, :],
                                w_t[:, ko, nh * 512:(nh + 1) * 512],
                                start=(ko == 0), stop=(ko == HO - 1))
                        dst = (st[:, nh * 512:(nh + 1) * 512] if do_rope
                               else a_nat[:, so, nh * 512:(nh + 1) * 512])
                        nc.vector.tensor_tensor(
                            dst, ps[:],
                            bb[:, nh * 512:(nh + 1) * 512], mybir.AluOpType.add)
                    if do_rope:
                        _rope_chunk(st, so)
                        nc.vector.tensor_copy(a_nat[:, so, :], st[:])
                return a_nat

            BAND = 12  # T[s', s] == 0 for |s' - s| > 11 (structural)

            def _band_sos(o0, o1):
                """so chunks whose s-range intersects [o0-BAND, o1+BAND)."""
                return [so for so in range(SO)
                        if so * P + P > o0 - BAND and so * P < o1 + BAND]

            def t_agg_spill(a_nat, tt, d_scr):
                """(T @ A).T evicted through SBUF chunks into DRAM scratch.
                Evictions ride the scalar engine -- idle during phase 1."""
                for hc in range(HO):
                    for sh in range(2):
                        sos = _band_sos(sh * 512, (sh + 1) * 512)
                        ps = mmps.tile([P, 512], F32, tag="mm")
                        for so in sos:
                            nc.tensor.matmul(
                                ps[:], a_nat[:, so, hc * P:(hc + 1) * P],
                                tt[:, so, sh * 512:(sh + 1) * 512],
                                start=(so == sos[0]), stop=(so == sos[-1]))
                        ev = s2.tile([P, 512], BF16, tag="s2")
                        nc.scalar.copy(ev[:], ps[:])
                        nc.sync.dma_start(
                            d_scr[hc].ap()[:, sh * 512:(sh + 1) * 512], ev[:])

            def t_agg_v(v_nat, tt):
                """V_ext [P, SO, NH, HD+1] (bf16) = T @ V with ones column."""
                v_ext = big.tile([P, SO, NH, HD + 1], BF16, tag="big")
                nc.vector.tensor_copy(
                    v_ext[:, :, :, HD:HD + 1],
                    ones[:, None, None, :].to_broadcast((P, SO, NH, 1)))
                for sc in range(SO):
                    sos = _band_sos(sc * P, (sc + 1) * P)
                    for dh in range(2):
                        ps = mmps.tile([P, 512], F32, tag="mm")
                        for so in sos:
                            nc.tensor.matmul(
                                ps[:], tt[:, so, sc * P:(sc + 1) * P],
                                v_nat[:, so, dh * 512:(dh + 1) * 512],
                                start=(so == sos[0]), stop=(so == sos[-1]))
                        pvw = ps[:].rearrange("p (nh d) -> p nh d", d=HD)
                        nc.scalar.copy(
                            v_ext[:, sc, dh * 8:(dh + 1) * 8, 0:HD], pvw)
                return v_ext

            # ---- phase 1: V, Q, K  (projection + RoPE + temporal aggregation)
            v_nat = project("v")
            tt = big.tile([P, SO, S], BF16, tag="big")
            nc.sync.dma_start(tt[:], d_tt.ap())
            v_ext = t_agg_v(v_nat, tt)

            q_nat = project("q", do_rope=True)
            t_agg_spill(q_nat, tt, d_qs)

            k_nat = project("k", do_rope=True)
            t_agg_spill(k_nat, tt, d_ks)

            # ---- phase 2: attention (normalization + LN stats fold into the
            # head loop so the tail barrier shrinks to the final LN apply)
            attn_T = big.tile([P, HO, S], F32, tag="big")
            acc = s4.tile([P, S], F32R, tag="s4")
            acc2 = s4.tile([P, S], F32R, tag="s4")
            rb_c = None
            for h in range(NH):
                hc, off = h // 2, (h % 2) * HD
                # zero-pad the contraction dim to K=128: half-array (K=64)
                # matmuls never trip the PE activity monitor, pinning the
                # clock at 1.2 GHz. Rows 64:128 come from a DRAM zeros pad.
                kh = s4.tile([P, S], BF16, tag="s4")
                nc.sync.dma_start(kh[0:HD, :], d_ks[hc].ap()[off:off + HD, :])
                nc.sync.dma_start(kh[HD:P, :], d_zp.ap())
                qh = s4.tile([P, S], BF16, tag="s4")
                nc.sync.dma_start(qh[0:HD, :], d_qs[hc].ap()[off:off + HD, :])
                nc.sync.dma_start(qh[HD:P, :], d_zp.ap())
                if off == 0:
                    rb_c = s4.tile([P, S], F32, tag="s4")
                for q2 in range(2):
                    pv = pvps.tile([P, 512], F32, tag="pv")
                    # software-pipelined: scores run one kc ahead of PV
                    ets = []
                    for kc in range(SO + 1):
                        if kc < SO:
                            sp = scps.tile([P, 512], F32, tag="sc")
                            nc.tensor.matmul(
                                sp[:], kh[0:P, kc * P:(kc + 1) * P],
                                qh[0:P, q2 * 512:(q2 + 1) * 512],
                                start=True, stop=True, skip_group_check=True)
                            e_t = s2.tile([P, 512], BF16, tag="s2")
                            nc.scalar.activation(
                                e_t[:], sp[:],
                                mybir.ActivationFunctionType.Exp, scale=0.125)
                            ets.append(e_t)
                        if kc > 0:
                            j = kc - 1
                            nc.tensor.matmul(
                                pv[0:HD + 1, :], v_ext[:, j, h, :], ets[j][:],
                                start=(j == 0), stop=(j == SO - 1),
                                skip_group_check=True)
                    # evict raw out + sums; broadcast sums (no PE dependency).
                    # partition_broadcast only writes reliably at partition 0,
                    # so odd heads bounce through a temp + DVE copy.
                    qs = slice(q2 * 512, (q2 + 1) * 512)
                    nc.vector.tensor_copy(attn_T[off:off + HD, hc, qs], pv[0:HD, :])
                    srow = s2.tile([1, 512], F32, tag="s2")
                    nc.vector.tensor_copy(srow[:], pv[HD:HD + 1, :])
                    if off == 0:
                        nc.gpsimd.partition_broadcast(rb_c[0:HD, qs], srow[:])
                    else:
                        tmp = s2.tile([HD, 512], F32, tag="s2")
                        nc.gpsimd.partition_broadcast(tmp[:], srow[:])
                        nc.vector.tensor_copy(rb_c[off:off + HD, qs], tmp[:])
                if off == HD:
                    # chunk hc complete: normalize + accumulate LN stats
                    rcp_c = s4.tile([P, S], F32, tag="s4")
                    nc.vector.reciprocal_approx_fast(rcp_c[:], rb_c[:])
                    nc.vector.tensor_tensor(attn_T[:, hc, :], attn_T[:, hc, :],
                                            rcp_c[:], mybir.AluOpType.mult)
                    if hc == 0:
                        nc.vector.tensor_copy(acc[:], attn_T[:, 0, :])
                        nc.vector.tensor_tensor(acc2[:], attn_T[:, 0, :],
                                                attn_T[:, 0, :],
                                                mybir.AluOpType.mult)
                    else:
                        nc.vector.tensor_tensor(acc[:], acc[:], attn_T[:, hc, :],
                                                mybir.AluOpType.add)
                        sqc = s4.tile([P, S], F32, tag="s4")
                        nc.vector.tensor_tensor(sqc[:], attn_T[:, hc, :],
                                                attn_T[:, hc, :],
                                                mybir.AluOpType.mult)
                        nc.vector.tensor_tensor(acc2[:], acc2[:], sqc[:],
                                                mybir.AluOpType.add)

            # prefetch out-projection weights so the DMA overlaps LayerNorm
            wo_t = big.tile([P, HO, H], BF16, tag="big")
            nc.sync.dma_start(wo_t[:], d_w["o"].ap())
            brow_o = s4.tile([1, H], F32, tag="s4")
            nc.sync.dma_start(brow_o[:], d_b["o"].ap())
            bo_b = s4.tile([P, H], F32, tag="s4")
            nc.gpsimd.partition_broadcast(bo_b[:], brow_o[:])

            # ---- phase 3: LayerNorm over h (partition axis across HO chunks)
            # partition sums via a PE ones-matmul (gpsimd allreduce is slow)
            ones_r = cpool.tile([P, 1], F32R, name="ones_r")
            nc.vector.tensor_copy(ones_r[:], ones[:])
            mu_b = s4.tile([P, S], F32, tag="s4")
            ms_b = s4.tile([P, S], F32, tag="s4")
            for src, dst in ((acc, mu_b), (acc2, ms_b)):
                for half in range(2):
                    pss = mmps.tile([P, 512], F32, tag="mm")
                    nc.tensor.matmul(pss[0:1, :], ones_r[:],
                                     src[:, half * 512:(half + 1) * 512],
                                     start=True, stop=True,
                                     skip_group_check=True)
                    srw = s2.tile([1, 512], F32, tag="s2")
                    nc.vector.tensor_copy(srw[:], pss[0:1, :])
                    nc.gpsimd.partition_broadcast(
                        dst[:, half * 512:(half + 1) * 512], srw[:])
            nc.vector.tensor_scalar_mul(mu_b[:], mu_b[:], 1.0 / H)
            nc.vector.tensor_scalar_mul(ms_b[:], ms_b[:], 1.0 / H)
            m2 = s4.tile([P, S], F32, tag="s4")
            nc.scalar.square(m2[:], mu_b[:])
            nc.vector.tensor_tensor(ms_b[:], ms_b[:], m2[:], mybir.AluOpType.subtract)
            nc.scalar.activation(ms_b[:], ms_b[:], mybir.ActivationFunctionType.Sqrt,
                                 bias=eps_t[:])
            rstd = s4.tile([P, S], F32, tag="s4")
            nc.vector.reciprocal_approx_fast(rstd[:], ms_b[:])

            ln_out = big.tile([P, HO, S], BF16, tag="big")
            for hc in range(HO):
                t1 = s4.tile([P, S], F32, tag="s4")
                nc.vector.tensor_tensor(t1[:], attn_T[:, hc, :], mu_b[:],
                                        mybir.AluOpType.subtract)
                nc.vector.tensor_tensor(t1[:], t1[:], rstd[:],
                                        mybir.AluOpType.mult)
                nc.vector.tensor_scalar(ln_out[:, hc, :], t1[:],
                                        gam_t[:, hc:hc + 1], bet_t[:, hc:hc + 1],
                                        mybir.AluOpType.mult, mybir.AluOpType.add)

            # ---- phase 4: output projection
            for so in range(SO):
                for nh in range(2):
                    ps = mmps.tile([P, 512], F32, tag="mm")
                    for hc in range(HO):
                        nc.tensor.matmul(
                            ps[:], ln_out[:, hc, so * P:(so + 1) * P],
                            wo_t[:, hc, nh * 512:(nh + 1) * 512],
                            start=(hc == 0), stop=(hc == HO - 1))
                    ych = s2.tile([P, 512], F32, tag="s2")
                    nc.vector.tensor_tensor(ych[:], ps[:],
                                            bo_b[:, nh * 512:(nh + 1) * 512],
                                            mybir.AluOpType.add)
                    nc.sync.dma_start(
                        d_y.ap()[:, so, nh * 512:(nh + 1) * 512], ych[:])

    nc.compile()
    return nc


_NC = None


def _get_nc():
    global _NC
    if _NC is None:
        _NC = _build_program()
    return _NC


def _host_inputs(query, key, value, Wq, bq, Wk, bk, Wv, bv, Wo, bo,
                 temporal_weights, ln_gamma, ln_beta):
    T = _temporal_matrix(temporal_weights)
    tt_host = np.ascontiguousarray(  # TT[p, so, s'] = T[s', so*P+p]
        T.T.reshape(SO, P, S).transpose(1, 0, 2)).astype(NPBF16)
    cos, sin = _rope_tables()
    common = {
        "w_v": _nat(np.asarray(Wv, np.float32)).astype(NPBF16),
        "w_q": _nat(np.asarray(Wq, np.float32)).astype(NPBF16),
        "w_k": _nat(np.asarray(Wk, np.float32)).astype(NPBF16),
        "w_o": _nat(np.asarray(Wo, np.float32)).astype(NPBF16),
        "b_v": np.asarray(bv, np.float32).reshape(1, H),
        "b_q": np.asarray(bq, np.float32).reshape(1, H),
        "b_k": np.asarray(bk, np.float32).reshape(1, H),
        "b_o": np.asarray(bo, np.float32).reshape(1, H),
        "tt": tt_host,
        "zpad": np.zeros((HD, S), NPBF16),
        "cos_t": _nat(cos),
        "sin_t": _nat(sin),
        "ln_g": np.ascontiguousarray(
            np.asarray(ln_gamma, np.float32).reshape(HO, P).T),
        "ln_b": np.ascontiguousarray(
            np.asarray(ln_beta, np.float32).reshape(HO, P).T),
    }
    in_maps = []
    for c in range(N_CORES):
        m = dict(common)
        m["xt_q"] = _xt_chunks(np.asarray(query[c], np.float32)).astype(NPBF16)
        m["xt_k"] = _xt_chunks(np.asarray(key[c], np.float32)).astype(NPBF16)
        m["xt_v"] = _xt_chunks(np.asarray(value[c], np.float32)).astype(NPBF16)
        in_maps.append(m)
    return in_maps


def kernel(query, key, value, Wq, bq, Wk, bk, Wv, bv, Wo, bo,
           temporal_weights, ln_gamma, ln_beta):
    in_maps = _host_inputs(query, key, value, Wq, bq, Wk, bk, Wv, bv, Wo, bo,
                           temporal_weights, ln_gamma, ln_beta)
    nc = _get_nc()
    res = run_bass_kernel_spmd(nc, in_maps, list(range(N_CORES)))
    out = np.empty((B, S, H), np.float32)
    for c in range(N_CORES):
        y = res.results[c]["y"]  # [P, SO, H]
        out[c] = y.transpose(1, 0, 2).reshape(S, H)
    return out


# revision 12
# speedup vs baseline: 1.2072x; 1.1161x over previous
"""HSTU-style attention block (RoPE + multi-scale temporal agg + SDPA + LN + out-proj)
for Trainium2, data-parallel over batch across 8 NeuronCores.

Per-core layout strategy (batch element per core):
  - host pre-transposes X so projections run with activations as lhsT
  - Q/K/V projected into natural [s, h'] layout; RoPE applied in-place in bf16
    (all-bf16 packed operands ride the DVE 2x mode)
  - temporal aggregation applied as a matmul against a host-built [S, S] matrix T
    (softmax(temporal_weights)); band structure (|s'-s| <= 11) trims contraction
    chunks at 256-wide output granularity; Q/K produced transposed, V natural
    with an extra ones column so softmax denominators ride the PV matmul
  - attention computes scores^T per head (contraction zero-padded to K=128),
    Exp on the scalar engine two chunks ahead of the PV accumulation
  - LayerNorm statistics are taken in the transposed [s-partition] layout with
    tiny N=1 ones-matmuls (deferred one chunk so the PE never waits on DVE);
    gamma/beta fold into the out-projection weights on the host, so the LN
    apply collapses to a per-partition scale at PSUM eviction plus a rank-1
    correction (mu*rstd) x colsum(gamma*Wo)
All matmuls run in bfloat16 (fp32 PSUM accumulation).
"""

import numpy as np
import ml_dtypes
import concourse.mybir as mybir
import concourse.tile as tile
from concourse import bacc
from concourse.bass_utils import run_bass_kernel_spmd

B, S, H, NH = 8, 1024, 1024, 16
HD = H // NH  # 64
P = 128
SO = S // P  # 8
HO = H // P  # 8
N_SCALES = 4
LN_EPS = 1e-5
F32 = mybir.dt.float32
BF16 = mybir.dt.bfloat16
NPBF16 = ml_dtypes.bfloat16

N_CORES = 8
AHEAD = 2  # exp pipeline depth (score chunks ahead of PV)


# ---------------------------------------------------------------- host helpers
def _softmax_np(x):
    x = np.asarray(x, np.float64)
    e = np.exp(x - x.max())
    return e / e.sum()


def _temporal_matrix(temporal_weights):
    """[S, S] matrix T with (T @ x) == temporal_agg(x) along the sequence axis."""
    w = _softmax_np(temporal_weights)
    T = np.eye(S, dtype=np.float64) * w[0]
    for scale in range(1, N_SCALES):
        p = max(1, S // (2 ** scale))
        k = S // p
        pool = np.zeros((p, S), dtype=np.float64)
        for j in range(p):
            pool[j, j * k:(j + 1) * k] = 1.0 / k
        coord = (np.arange(S, dtype=np.float64) + 0.5) * (p / S) - 0.5
        coord = np.clip(coord, 0.0, None)
        i0 = np.minimum(np.floor(coord).astype(np.int64), p - 1)
        i1 = np.minimum(i0 + 1, p - 1)
        lam = (coord - i0).astype(np.float32).astype(np.float64)
        interp = np.zeros((S, p), dtype=np.float64)
        interp[np.arange(S), i0] += 1.0 - lam
        interp[np.arange(S), i1] += lam
        T += w[scale] * (interp @ pool)
    return T.astype(np.float32)


def _rope_tables():
    inv_freq = 1.0 / (10000.0 ** (np.arange(0, HD, 2, dtype=np.float64) / HD))
    freqs = np.arange(S, dtype=np.float64)[:, None] * inv_freq[None, :]
    cos = np.repeat(np.cos(freqs), 2, axis=-1).astype(np.float32)  # [S, HD]
    sin = np.repeat(np.sin(freqs), 2, axis=-1).astype(np.float32)
    return cos, sin


def _nat(x):
    """[S, D] -> [P, S//P, D] with x[so*P+p, d] = out[p, so, d]."""
    return np.ascontiguousarray(x.reshape(SO, P, x.shape[-1]).transpose(1, 0, 2))


def _xt_chunks(x):
    """[S, H] -> [P, SO, HO*P] with out[p, so, ho*P + i] = x[so*P + i, ho*P + p]."""
    return np.ascontiguousarray(
        x.reshape(SO, P, HO, P).transpose(3, 0, 2, 1).reshape(P, SO, H))


# ---------------------------------------------------------------- bass program
def _build_program():
    nc = bacc.Bacc("TRN2", target_bir_lowering=False, debug=False)

    d_xt = {a: nc.dram_tensor(f"xt_{a}", [P, SO, H], BF16, kind="ExternalInput")
            for a in ("v", "q", "k")}
    d_w = {a: nc.dram_tensor(f"w_{a}", [P, HO, H], BF16, kind="ExternalInput")
           for a in ("v", "q", "k", "o")}
    d_b = {a: nc.dram_tensor(f"b_{a}", [1, H], F32, kind="ExternalInput")
           for a in ("v", "q", "k", "o")}
    d_g1n = nc.dram_tensor("g1n", [1, H], F32, kind="ExternalInput")
    d_tt = nc.dram_tensor("tt", [P, SO, S], BF16, kind="ExternalInput")
    d_cos = nc.dram_tensor("cos_t", [P, SO, HD], BF16, kind="ExternalInput")
    d_sin = nc.dram_tensor("sin_t", [P, SO, HD], BF16, kind="ExternalInput")
    d_y = nc.dram_tensor("y", [P, SO, H], F32, kind="ExternalOutput")
    d_zp = nc.dram_tensor("zpad", [HD, S], BF16, kind="ExternalInput")
    # per-chunk scratch so a head's reload only waits on its own spill DMA
    d_qs = [nc.dram_tensor(f"q_scr{hc}", [P, S], BF16) for hc in range(HO)]
    d_ks = [nc.dram_tensor(f"k_scr{hc}", [P, S], BF16) for hc in range(HO)]

    with tile.TileContext(nc) as tc:
        with (
            tc.tile_pool(name="const", bufs=1) as cpool,
            tc.tile_pool(name="big", bufs=4) as big,
            tc.tile_pool(name="s4", bufs=12) as s4,
            tc.tile_pool(name="s2", bufs=8) as s2,
            tc.tile_pool(name="sq", bufs=2) as sqp,
            tc.tile_pool(name="mm_ps", bufs=2, space="PSUM") as mmps,
            tc.tile_pool(name="sc_ps", bufs=3, space="PSUM") as scps,
            tc.tile_pool(name="pv_ps", bufs=2, space="PSUM") as pvps,
            tc.tile_pool(name="st_ps", bufs=1, space="PSUM") as stps,
        ):
            cos_t = cpool.tile([P, SO, HD], BF16, name="cos_t")
            sin_t = cpool.tile([P, SO, HD], BF16, name="sin_t")
            nc.sync.dma_start(cos_t[:], d_cos.ap())
            nc.sync.dma_start(sin_t[:], d_sin.ap())
            ones = cpool.tile([P, 1], F32, name="ones")
            nc.vector.memset(ones[:], 1.0)
            eps_t = cpool.tile([P, 1], F32, name="eps_t")
            nc.vector.memset(eps_t[:], LN_EPS)
            # stats rhs: 1/H so the PSUM accumulators hold means directly
            ones_st = cpool.tile([P, 1], BF16, name="ones_st")
            nc.vector.memset(ones_st[:], 1.0 / H)

            def _rope_chunk(a_nat, so):
                """In-place bf16 RoPE on a_nat[:, so, :] (DVE 2x mode)."""
                ch = a_nat[:, so, :]
                ch3 = ch.rearrange("p (nh d) -> p nh d", d=HD)
                ch4 = ch.rearrange("p (nh hf dd) -> p nh hf dd", hf=2, dd=HD // 2)
                rot = s4.tile([P, H], BF16, tag="s4")
                rot4 = rot[:].rearrange("p (nh hf dd) -> p nh hf dd",
                                        hf=2, dd=HD // 2)
                rot3 = rot[:].rearrange("p (nh d) -> p nh d", d=HD)
                nc.vector.tensor_scalar_mul(rot4[:, :, 0, :], ch4[:, :, 1, :], -1.0)
                nc.vector.tensor_copy(rot4[:, :, 1, :], ch4[:, :, 0, :])
                cb = cos_t[:, so, :][:, None, :].to_broadcast((P, NH, HD))
                sb = sin_t[:, so, :][:, None, :].to_broadcast((P, NH, HD))
                nc.vector.tensor_tensor(ch3[:], ch3[:], cb, mybir.AluOpType.mult)
                nc.vector.tensor_tensor(rot3[:], rot3[:], sb, mybir.AluOpType.mult)
                nc.vector.tensor_tensor(ch[:], ch[:], rot[:], mybir.AluOpType.add)

            def project(a, do_rope=False):
                """A_nat [P, SO, H] (bf16) = X @ W_a + b_a, optional fused RoPE."""
                w_t = big.tile([P, HO, H], BF16, tag="big")
                nc.sync.dma_start(w_t[:], d_w[a].ap())
                brow = s4.tile([1, H], F32, tag="s4")
                nc.sync.dma_start(brow[:], d_b[a].ap())
                bb = s4.tile([P, H], F32, tag="s4")
                nc.gpsimd.partition_broadcast(bb[:], brow[:])
                a_nat = big.tile([P, SO, H], BF16, tag="big")
                for so in range(SO):
                    xt_c = s4.tile([P, HO, P], BF16, tag="s4")
                    nc.sync.dma_start(xt_c[:], d_xt[a].ap()[:, so, :])
                    for nh in range(2):
                        ps = mmps.tile([P, 512], F32, tag="mm")
                        for ko in range(HO):
                            nc.tensor.matmul(
                                ps[:], xt_c[:, ko, :],
                                w_t[:, ko, nh * 512:(nh + 1) * 512],
                                start=(ko == 0), stop=(ko == HO - 1))
                        nc.vector.tensor_tensor(
                            a_nat[:, so, nh * 512:(nh + 1) * 512], ps[:],
                            bb[:, nh * 512:(nh + 1) * 512], mybir.AluOpType.add)
                    if do_rope:
                        _rope_chunk(a_nat, so)
                return a_nat

            BAND = 12  # T[s', s] == 0 for |s' - s| > 11 (structural)

            def _band_sos(o0, o1):
                """so chunks whose s-range intersects [o0-BAND, o1+BAND)."""
                return [so for so in range(SO)
                        if so * P + P > o0 - BAND and so * P < o1 + BAND]

            def t_agg_spill(a_nat, tt, d_scr, cb=None):
                """(T @ A).T evicted through SBUF chunks into DRAM scratch.
                256-wide output slices keep the banded contraction tight."""
                for hc in range(HO):
                    if cb is not None:
                        cb(hc)
                    for sh in range(2):
                        ps = mmps.tile([P, 512], F32, tag="mm")
                        for q in range(2):
                            o0 = sh * 512 + q * 256
                            sos = _band_sos(o0, o0 + 256)
                            for so in sos:
                                nc.tensor.matmul(
                                    ps[:, q * 256:(q + 1) * 256],
                                    a_nat[:, so, hc * P:(hc + 1) * P],
                                    tt[:, so, o0:o0 + 256],
                                    start=(so == sos[0]), stop=(so == sos[-1]),
                                    skip_group_check=True)
                        ev = s2.tile([P, 512], BF16, tag="s2")
                        nc.scalar.copy(ev[:], ps[:])
                        nc.sync.dma_start(
                            d_scr[hc].ap()[:, sh * 512:(sh + 1) * 512], ev[:])

            def t_agg_v(v_nat, tt):
                """V_ext [P, SO, NH, HD+1] (bf16) = T @ V with ones column."""
                v_ext = big.tile([P, SO, NH, HD + 1], BF16, tag="big")
                nc.vector.tensor_copy(
                    v_ext[:, :, :, HD:HD + 1],
                    ones[:, None, None, :].to_broadcast((P, SO, NH, 1)))
                for sc in range(SO):
                    sos = _band_sos(sc * P, (sc + 1) * P)
                    for dh in range(2):
                        ps = mmps.tile([P, 512], F32, tag="mm")
                        for so in sos:
                            nc.tensor.matmul(
                                ps[:], tt[:, so, sc * P:(sc + 1) * P],
                                v_nat[:, so, dh * 512:(dh + 1) * 512],
                                start=(so == sos[0]), stop=(so == sos[-1]))
                        pvw = ps[:].rearrange("p (nh d) -> p nh d", d=HD)
                        nc.scalar.copy(
                            v_ext[:, sc, dh * 8:(dh + 1) * 8, 0:HD], pvw)
                return v_ext

            # ---- phase 1: V, Q, K  (projection + RoPE + temporal aggregation)
            v_nat = project("v")
            tt = big.tile([P, SO, S], BF16, tag="big")
            nc.sync.dma_start(tt[:], d_tt.ap())
            v_ext = t_agg_v(v_nat, tt)

            q_nat = project("q", do_rope=True)
            t_agg_spill(q_nat, tt, d_qs)

            k_nat = project("k", do_rope=True)

            # prefetch the first two heads' K/Q reloads as soon as their
            # scratch chunk (hc=0) is spilled, so phase 2 starts immediately
            pre_kq = {}

            def _load_head(h):
                off = (h % 2) * HD
                hc = h // 2
                kh = s4.tile([P, S], BF16, tag="s4", name=f"kh{h}")
                nc.sync.dma_start(kh[0:HD, :], d_ks[hc].ap()[off:off + HD, :])
                nc.sync.dma_start(kh[HD:P, :], d_zp.ap())
                qh = s4.tile([P, S], BF16, tag="s4", name=f"qh{h}")
                nc.sync.dma_start(qh[0:HD, :], d_qs[hc].ap()[off:off + HD, :])
                nc.sync.dma_start(qh[HD:P, :], d_zp.ap())
                return kh, qh

            def _k_cb(hc):
                if hc == 1:
                    pre_kq[0] = _load_head(0)
                    pre_kq[1] = _load_head(1)

            t_agg_spill(k_nat, tt, d_ks, cb=_k_cb)

            # prefetch out-projection weights + folded LN rows during phase 2
            wo_t = big.tile([P, HO, H], BF16, tag="big")
            nc.sync.dma_start(wo_t[:], d_w["o"].ap())
            b1r = s4.tile([1, H], F32, tag="s4")
            nc.sync.dma_start(b1r[:], d_b["o"].ap())
            b1b = cpool.tile([P, H], F32, name="b1b")
            nc.gpsimd.partition_broadcast(b1b[:], b1r[:])
            g1r = s4.tile([1, H], F32, tag="s4")
            nc.sync.dma_start(g1r[:], d_g1n.ap())
            g1nb = cpool.tile([P, H], F32, name="g1nb")
            nc.gpsimd.partition_broadcast(g1nb[:], g1r[:])

            # ---- phase 2: attention; softmax normalization + transposed LN
            # stats (s on partitions) fold into the head loop
            attn_T = big.tile([P, HO, S], BF16, tag="big")
            # per-(hc,so) single-shot stats columns: a start=True matmul
            # resets accumulate-state for its whole PSUM bank, so interleaved
            # accumulation chains in one bank are NOT safe -- write each
            # column exactly once and reduce across hc on the DVE instead
            stat_ps = stps.tile([P, HO * 16], F32, tag="stat")
            sqs = {}

            def _stats(hc):
                # mean / mean-square contributions of chunk hc (N=1 matmuls)
                for so in range(SO):
                    nc.tensor.matmul(
                        stat_ps[:, hc * 16 + so:hc * 16 + so + 1],
                        attn_T[:, hc, so * P:(so + 1) * P], ones_st[:],
                        start=True, stop=True, skip_group_check=True)
                    nc.tensor.matmul(
                        stat_ps[:, hc * 16 + 8 + so:hc * 16 + 9 + so],
                        sqs[hc][:, so * P:(so + 1) * P], ones_st[:],
                        start=True, stop=True, skip_group_check=True)

            rb_c = None
            for h in range(NH):
                hc, off = h // 2, (h % 2) * HD
                # zero-pad the contraction dim to K=128 (rows 64:128 from a
                # DRAM zeros pad) to keep the PE activity profile flat
                if h in pre_kq:
                    kh, qh = pre_kq[h]
                else:
                    kh, qh = _load_head(h)
                if off == 0:
                    rb_c = s4.tile([P, S], F32, tag="s4")
                for q2 in range(2):
                    pv = pvps.tile([P, 512], F32, tag="pv")
                    # software-pipelined: scores run AHEAD kc chunks ahead of
                    # PV so the scalar-engine Exp never stalls the PE
                    ets = []
                    for kc in range(SO + AHEAD):
                        if kc < SO:
                            sp = scps.tile([P, 512], F32, tag="sc")
                            nc.tensor.matmul(
                                sp[:], kh[0:P, kc * P:(kc + 1) * P],
                                qh[0:P, q2 * 512:(q2 + 1) * 512],
                                start=True, stop=True, skip_group_check=True)
                            e_t = s2.tile([P, 512], BF16, tag="s2")
                            nc.scalar.activation(
                                e_t[:], sp[:],
                                mybir.ActivationFunctionType.Exp, scale=0.125)
                            ets.append(e_t)
                        if kc >= AHEAD:
                            j = kc - AHEAD
                            nc.tensor.matmul(
                                pv[0:HD + 1, :], v_ext[:, j, h, :], ets[j][:],
                                start=(j == 0), stop=(j == SO - 1),
                                skip_group_check=True)
                    # evict raw out + sums; broadcast sums (no PE dependency).
                    # partition_broadcast only writes reliably at partition 0,
                    # so odd heads bounce through a temp + DVE copy.
                    qs = slice(q2 * 512, (q2 + 1) * 512)
                    nc.vector.tensor_copy(attn_T[off:off + HD, hc, qs], pv[0:HD, :])
                    srow = s2.tile([1, 512], F32, tag="s2")
                    nc.vector.tensor_copy(srow[:], pv[HD:HD + 1, :])
                    if off == 0:
                        nc.gpsimd.partition_broadcast(rb_c[0:HD, qs], srow[:])
                    else:
                        tmp = s2.tile([HD, 512], F32, tag="s2")
                        nc.gpsimd.partition_broadcast(tmp[:], srow[:])
                        nc.vector.tensor_copy(rb_c[off:off + HD, qs], tmp[:])
                if off == HD:
                    # chunk hc complete. Issue the PREVIOUS chunk's stats
                    # matmuls first -- before this chunk's DVE chain is even
                    # queued -- so the PE stream never picks up a false
                    # dependency on this boundary's normalize/square work.
                    if hc >= 1:
                        _stats(hc - 1)
                    rcp_c = s4.tile([P, S], F32, tag="s4")
                    nc.vector.reciprocal_approx_fast(rcp_c[:], rb_c[:])
                    nc.vector.tensor_tensor(attn_T[:, hc, :], attn_T[:, hc, :],
                                            rcp_c[:], mybir.AluOpType.mult)
                    sq_c = sqp.tile([P, S], BF16, tag="sq")
                    nc.vector.tensor_tensor(sq_c[:], attn_T[:, hc, :],
                                            attn_T[:, hc, :],
                                            mybir.AluOpType.mult)
                    sqs[hc] = sq_c
            _stats(HO - 1)

            # ---- phase 3: LN scale factors in the transposed layout ([P, 8],
            # partition = s within chunk, free = so) -- all tiny ops
            st3 = stat_ps[:].rearrange("p (hc c) -> p hc c", c=16)
            acc16 = s2.tile([P, 16], F32, tag="s2")
            nc.vector.tensor_copy(acc16[:], st3[:, 0, :])
            for hc in range(1, HO):
                nc.vector.tensor_tensor(acc16[:], acc16[:], st3[:, hc, :],
                                        mybir.AluOpType.add)
            m2 = s2.tile([P, 8], F32, tag="s2")
            nc.scalar.square(m2[:], acc16[:, 0:8])
            var_t = s2.tile([P, 8], F32, tag="s2")
            nc.vector.tensor_tensor(var_t[:], acc16[:, 8:16], m2[:],
                                    mybir.AluOpType.subtract)
            nc.scalar.activation(var_t[:], var_t[:],
                                 mybir.ActivationFunctionType.Sqrt, bias=eps_t[:])
            rstd = s2.tile([P, 8], F32, tag="s2")
            nc.vector.reciprocal_approx_fast(rstd[:], var_t[:])
            rmu = s2.tile([P, 8], F32, tag="s2")
            nc.vector.tensor_tensor(rmu[:], acc16[:, 0:8], rstd[:],
                                    mybir.AluOpType.mult)

            # ---- phase 4: output projection on raw attn with fused LN:
            # y = rstd*(attn^T @ W') - (mu*rstd)*g1 + b1
            for so in range(SO):
                for nh in range(2):
                    t2 = s2.tile([P, 512], F32, tag="s2")
                    nc.vector.scalar_tensor_tensor(
                        t2[:], g1nb[:, nh * 512:(nh + 1) * 512],
                        rmu[:, so:so + 1], b1b[:, nh * 512:(nh + 1) * 512],
                        mybir.AluOpType.mult, mybir.AluOpType.add)
                    ps = mmps.tile([P, 512], F32, tag="mm")
                    for hc in range(HO):
                        nc.tensor.matmul(
                            ps[:], attn_T[:, hc, so * P:(so + 1) * P],
                            wo_t[:, hc, nh * 512:(nh + 1) * 512],
                            start=(hc == 0), stop=(hc == HO - 1))
                    ych = s2.tile([P, 512], F32, tag="s2")
                    nc.scalar.activation(ych[:], ps[:],
                                         mybir.ActivationFunctionType.Copy,
                                         scale=rstd[:, so:so + 1])
                    nc.vector.tensor_tensor(ych[:], ych[:], t2[:],
                                            mybir.AluOpType.add)
                    nc.sync.dma_start(
                        d_y.ap()[:, so, nh * 512:(nh + 1) * 512], ych[:])

    nc.compile()
    return nc


_NC = None


def _get_nc():
    global _NC
    if _NC is None:
        _NC = _build_program()
    return _NC


def _host_inputs(query, key, value, Wq, bq, Wk, bk, Wv, bv, Wo, bo,
                 temporal_weights, ln_gamma, ln_beta):
    T = _temporal_matrix(temporal_weights)
    tt_host = np.ascontiguousarray(  # TT[p, so, s'] = T[s', so*P+p]
        T.T.reshape(SO, P, S).transpose(1, 0, 2)).astype(NPBF16)
    cos, sin = _rope_tables()
    gam = np.asarray(ln_gamma, np.float32)
    bet = np.asarray(ln_beta, np.float32)
    Wo32 = np.asarray(Wo, np.float32)
    Wfold = gam[:, None] * Wo32               # gamma folded into out-proj
    g1n = -(gam @ Wo32).reshape(1, H)         # rank-1 LN correction row
    b1 = (bet @ Wo32 + np.asarray(bo, np.float32)).reshape(1, H)
    common = {
        "w_v": _nat(np.asarray(Wv, np.float32)).astype(NPBF16),
        "w_q": _nat(np.asarray(Wq, np.float32)).astype(NPBF16),
        "w_k": _nat(np.asarray(Wk, np.float32)).astype(NPBF16),
        "w_o": _nat(Wfold).astype(NPBF16),
        "b_v": np.asarray(bv, np.float32).reshape(1, H),
        "b_q": np.asarray(bq, np.float32).reshape(1, H),
        "b_k": np.asarray(bk, np.float32).reshape(1, H),
        "b_o": b1,
        "g1n": g1n,
        "tt": tt_host,
        "zpad": np.zeros((HD, S), NPBF16),
        "cos_t": _nat(cos).astype(NPBF16),
        "sin_t": _nat(sin).astype(NPBF16),
    }
    in_maps = []
    for c in range(N_CORES):
        m = dict(common)
        m["xt_q"] = _xt_chunks(np.asarray(query[c], np.float32)).astype(NPBF16)
        m["xt_k"] = _xt_chunks(np.asarray(key[c], np.float32)).astype(NPBF16)
        m["xt_v"] = _xt_chunks(np.asarray(value[c], np.float32)).astype(NPBF16)
        in_maps.append(m)
    return in_maps


def kernel(query, key, value, Wq, bq, Wk, bk, Wv, bv, Wo, bo,
           temporal_weights, ln_gamma, ln_beta):
    in_maps = _host_inputs(query, key, value, Wq, bq, Wk, bk, Wv, bv, Wo, bo,
                           temporal_weights, ln_gamma, ln_beta)
    nc = _get_nc()
    res = run_bass_kernel_spmd(nc, in_maps, list(range(N_CORES)))
    out = np.empty((B, S, H), np.float32)
    for c in range(N_CORES):
        y = res.results[c]["y"]  # [P, SO, H]
        out[c] = y.transpose(1, 0, 2).reshape(S, H)
    return out
